# revision 1
# baseline (speedup 1.0000x reference)
"""Document-causal GQA attention on 8 TRN2 NeuronCores.

Strategy: the packed-document mask makes attention block-diagonal over
(batch, document) segments, so each of the 8 cores gets one segment's
queries (2 batches x ~4 docs) together with its KV window — no
cross-core communication at all. The host shards/transposes inputs,
each core runs the full QKV->RoPE->softmax->PV->Wo pipeline on its
rows, and the host scatters the disjoint output rows back.

Device kernel (SPMD, one graph): bf16 matmuls (FWL-eligible 128x128
stationary operands via a head permutation + zero-padded K/V weights),
fp32 PSUM, ACT exp with folded 1/sqrt(hd) scale, diagonal-block-only
masking in the pure-causal case, softmax denominators via a ones
column appended to V, batched reciprocal + ones-outer-product
broadcast for normalization.
"""
import numpy as np
import ml_dtypes

from contextlib import ExitStack

import concourse.bass as bass
import concourse.tile as tile
from concourse import bacc, mybir
from concourse.bass_utils import run_bass_kernel_spmd

BS, S, D, H, KVH, HD = 2, 2048, 2048, 32, 8, 64
N_REP = H // KVH
HQ = H * HD
HKV = KVH * HD
P = 128
N_CORES = 8
DT = D // P
HC = HQ // 512
HQT = HQ // P

f32 = mybir.dt.float32
bf16 = mybir.dt.bfloat16
EXPF = mybir.ActivationFunctionType.Exp
bf = ml_dtypes.bfloat16

HEAD_ORDER = [i // 2 if i % 2 == 0 else 16 + i // 2 for i in range(32)]


# ---------------------------------------------------------------------------
# host-side planning
# ---------------------------------------------------------------------------

def _round_up(x, m):
    return ((x + m - 1) // m) * m


def _plan_jobs(sequence_id):
    jobs = []
    for b in range(BS):
        sid = np.asarray(sequence_id[b])
        starts = [0] + list(np.where(np.diff(sid) != 0)[0] + 1) + [len(sid)]
        for i in range(len(starts) - 1):
            jobs.append([b, int(starts[i]), int(starts[i + 1] - starts[i]),
                         int(starts[i])])
    while len(jobs) > N_CORES:
        best, bi = None, -1
        for i in range(len(jobs) - 1):
            a, c = jobs[i], jobs[i + 1]
            if a[0] == c[0] and a[1] + a[2] == c[1]:
                cost = (c[1] + c[2]) - min(a[3], c[3])
                if best is None or cost < best:
                    best, bi = cost, i
        a, c = jobs[bi], jobs[bi + 1]
        jobs[bi] = [a[0], a[1], a[2] + c[2], min(a[3], c[3])]
        del jobs[bi + 1]
    while len(jobs) < N_CORES:
        i = max(range(len(jobs)), key=lambda j: jobs[j][2])
        b, qs, ql, ks = jobs[i]
        if ql < 2:
            jobs.append([b, qs, 0, qs])
            continue
        h = ql // 2
        jobs[i] = [b, qs, h, ks]
        jobs.insert(i + 1, [b, qs + h, ql - h, ks])
    return jobs


def _permute_wq(wq_t):
    return np.ascontiguousarray(
        wq_t.reshape(D, 32, 64)[:, HEAD_ORDER, :].reshape(D, HQ))


def _permute_wo(wo_t):
    return np.ascontiguousarray(
        wo_t.reshape(32, 64, D)[HEAD_ORDER].reshape(HQ, D))


def _core_inputs(job, NQ, NK, x, sequence_id, cos_tab, sin_tab):
    b, qs, ql, ks = job
    kl = qs + ql - ks

    xq_t = np.zeros((D, NQ), dtype=np.float32)
    xq_t[:, :ql] = x[b, qs:qs + ql].T
    xk_t = np.zeros((D, NK), dtype=np.float32)
    xk_t[:, :kl] = x[b, ks:ks + kl].T

    def rope(start, ln, n):
        cos = np.ones((n, 64), dtype=np.float32)
        sin = np.zeros((n, 64), dtype=np.float32)
        c = cos_tab[start:start + ln]
        s = sin_tab[start:start + ln]
        cos[:ln, 0::2] = c
        cos[:ln, 1::2] = c
        sin[:ln, 0::2] = -s
        sin[:ln, 1::2] = s
        return (np.tile(cos, (1, 8)).astype(bf), np.tile(sin, (1, 8)).astype(bf))

    cos_q, sin_q = rope(qs, ql, NQ)
    cos_k, sin_k = rope(ks, kl, NK)

    sid = np.asarray(sequence_id[b])
    sid_q = np.full(NQ, -2, dtype=np.int64)
    sid_q[:ql] = sid[qs:qs + ql]
    sid_k = np.full(NK, -1, dtype=np.int64)
    sid_k[:kl] = sid[ks:ks + kl]
    gq = qs + np.arange(NQ)
    gk = ks + np.arange(NK)
    mask = ((sid_k[:, None] == sid_q[None, :]) &
            (gk[:, None] <= gq[None, :])).astype(np.float32)
    # padded query columns attend to key 0 so denominators stay finite
    mask[0, ql:] = 1.0
    kones = np.zeros((NK, 1), dtype=np.float32)
    kones[:kl] = 1.0

    return {
        "xq_t": xq_t.astype(bf), "xk_t": xk_t.astype(bf),
        "cos_q": cos_q, "sin_q": sin_q, "cos_k": cos_k, "sin_k": sin_k,
        "maskm": mask.astype(bf), "kones": kones.astype(bf),
    }


# ---------------------------------------------------------------------------
# device graph
# ---------------------------------------------------------------------------

_BUILD_CACHE = {}


def _build(NQ, NK, offs_max, causal):
    key = (NQ, NK, offs_max, causal)
    if key in _BUILD_CACHE:
        return _BUILD_CACHE[key]
    NQT, NKT = NQ // P, NK // P
    qchunks = [(c * 512, min(512, NQ - c * 512)) for c in range((NQ + 511) // 512)]

    nc = bacc.Bacc("TRN2", target_bir_lowering=False, debug=False,
                   num_devices=N_CORES)

    xq_d = nc.dram_tensor("xq_t", [D, NQ], bf16, kind="ExternalInput").ap()
    xk_d = nc.dram_tensor("xk_t", [D, NK], bf16, kind="ExternalInput").ap()
    wq_d = nc.dram_tensor("wq_t", [D, HQ], bf16, kind="ExternalInput").ap()
    wk_d = nc.dram_tensor("wk_t", [D, HKV], bf16, kind="ExternalInput").ap()
    wv_d = nc.dram_tensor("wv_t", [D, HKV], bf16, kind="ExternalInput").ap()
    wo_d = nc.dram_tensor("wo_t", [HQ, D], bf16, kind="ExternalInput").ap()
    cosq_d = nc.dram_tensor("cos_q", [NQ, 512], bf16, kind="ExternalInput").ap()
    sinq_d = nc.dram_tensor("sin_q", [NQ, 512], bf16, kind="ExternalInput").ap()
    cosk_d = nc.dram_tensor("cos_k", [NK, 512], bf16, kind="ExternalInput").ap()
    sink_d = nc.dram_tensor("sin_k", [NK, 512], bf16, kind="ExternalInput").ap()
    mask_d = nc.dram_tensor("maskm", [NK, NQ], bf16, kind="ExternalInput").ap()
    kones_d = nc.dram_tensor("kones", [NK, 1], bf16, kind="ExternalInput").ap()
    id_d = nc.dram_tensor("ident", [P, P], bf16, kind="ExternalInput").ap()
    out_d = nc.dram_tensor("out", [NQ, HQ], f32, kind="ExternalOutput").ap()

    with tile.TileContext(nc) as tc, ExitStack() as ctx:
        const = ctx.enter_context(tc.tile_pool(name="const", bufs=1))
        persist = ctx.enter_context(tc.tile_pool(name="persist", bufs=1))
        xpool = ctx.enter_context(tc.tile_pool(name="xpool", bufs=2))
        wstream = ctx.enter_context(tc.tile_pool(name="wstream", bufs=2))
        work = ctx.enter_context(tc.tile_pool(name="work", bufs=2))
        ropetab = ctx.enter_context(tc.tile_pool(name="ropetab", bufs=1))
        pmpool = ctx.enter_context(tc.tile_pool(name="pmpool", bufs=8))
        pp = ctx.enter_context(tc.tile_pool(name="pp", bufs=2, space="PSUM"))
        psc = ctx.enter_context(tc.tile_pool(name="psc", bufs=3, space="PSUM"))
        pv = ctx.enter_context(tc.tile_pool(name="pv", bufs=3, space="PSUM"))

        # ---- initial loads: x/w chunks first so the PE starts ASAP ----
        xk_sb = xpool.tile([P, DT, NK], bf16, name="xsb")
        xk_r = xk_d.rearrange("(t p) q -> p t q", p=P)
        wkc = wstream.tile([P, DT, 512], bf16, name="wchunk")
        wk_r = wk_d.rearrange("(t p) o -> p t o", p=P)
        for c in range(4):
            nc.gpsimd.dma_start(xk_sb[:, 4 * c:4 * (c + 1), :],
                                xk_r[:, 4 * c:4 * (c + 1), :])
            nc.gpsimd.dma_start(wkc[:, 4 * c:4 * (c + 1), :],
                                wk_r[:, 4 * c:4 * (c + 1), :])

        ident = const.tile([P, P], bf16, name="ident")
        nc.gpsimd.dma_start(ident[:], id_d)
        ones64 = const.tile([1, HD], bf16, name="ones64")
        nc.vector.memset(ones64[:], 1.0)

        Qt = persist.tile([P, HQT, NQ], bf16, name="Qt")
        KtRz = persist.tile([P, KVH, NK], bf16, name="KtRz")
        Vaug = persist.tile([P, NKT, KVH, P], bf16, name="Vaug")
        attnT = persist.tile([P, HQT, NQ], bf16, name="attnT")
        mask_sb = persist.tile([P, NKT, NQ], bf16, name="mask_sb")

        nc.vector.memset(KtRz[64:128, 0:4, :], 0.0)
        nc.vector.memset(KtRz[0:64, 4:8, :], 0.0)
        nc.vector.memset(Vaug[:, :, :, HD:P], 0.0)
        kones_sb = const.tile([P, NKT], bf16, name="kones_sb")
        nc.gpsimd.dma_start(kones_sb[:], kones_d.rearrange("(t p) o -> p (t o)", p=P))
        for kt in range(NKT):
            for g in range(KVH):
                nc.vector.tensor_copy(Vaug[:, kt, g, HD:HD + 1],
                                      kones_sb[:, kt:kt + 1])

        cosk = ropetab.tile([P, NKT, 512], bf16, name="cos")
        sink = ropetab.tile([P, NKT, 512], bf16, name="sin")
        nc.gpsimd.dma_start(cosk[:], cosk_d.rearrange("(t p) c -> p t c", p=P))
        nc.gpsimd.dma_start(sink[:], sink_d.rearrange("(t p) c -> p t c", p=P))
        nc.gpsimd.dma_start(mask_sb[:], mask_d.rearrange("(t p) q -> p t q", p=P))

        def rope_block(ps, cos_t, sin_t, ti):
            nat = work.tile([P, 512], f32, name="nat")
            nc.vector.tensor_copy(nat[:], ps[:])
            ro = work.tile([P, 512], f32, name="ro")
            nc.gpsimd.tensor_mul(ro[:, 0::2], nat[:, 1::2], sin_t[:, ti, 0::2])
            nc.gpsimd.tensor_mul(ro[:, 1::2], nat[:, 0::2], sin_t[:, ti, 1::2])
            tmp = work.tile([P, 512], f32, name="tmp")
            nc.vector.tensor_mul(tmp[:], nat[:], cos_t[:, ti, :])
            rot = work.tile([P, 512], bf16, name="rot")
            nc.vector.tensor_add(rot[:], ro[:], tmp[:])
            return rot

        # ---- K projection + rope + transpose (zero-padded halves) ----
        for kt in range(NKT):
            ps = pp.tile([P, 512], f32, name="pj")
            for dt in range(DT):
                nc.tensor.matmul(ps[:], xk_sb[:, dt, kt * P:(kt + 1) * P],
                                 wkc[:, dt, :], start=(dt == 0),
                                 stop=(dt == DT - 1))
            rot = rope_block(ps, cosk, sink, kt)
            ks = slice(kt * P, (kt + 1) * P)
            for b in range(4):
                pst = psc.tile([P, P], bf16, name="psS")
                nc.tensor.transpose(pst[:], rot[:, b * P:(b + 1) * P], ident[:])
                half = (2 * b) // 4
                lo = half * 64
                nc.scalar.copy(KtRz[lo:lo + 64, 2 * b, ks], pst[0:64, :])
                nc.scalar.copy(KtRz[lo:lo + 64, 2 * b + 1, ks], pst[64:128, :])

        # ---- V projection -> Vaug ----
        wvc = wstream.tile([P, DT, 512], bf16, name="wchunk")
        nc.gpsimd.dma_start(wvc[:], wv_d.rearrange("(t p) o -> p t o", p=P))
        for kt in range(NKT):
            ps = pp.tile([P, 512], f32, name="pj")
            for dt in range(DT):
                nc.tensor.matmul(ps[:], xk_sb[:, dt, kt * P:(kt + 1) * P],
                                 wvc[:, dt, :], start=(dt == 0),
                                 stop=(dt == DT - 1))
            nc.vector.tensor_copy(Vaug[:, kt, :, 0:HD],
                                  ps[:].rearrange("p (g d) -> p g d", g=KVH))

        # ---- Q projection + rope + transpose (head-permuted wq) ----
        cosq = ropetab.tile([P, NQT, 512], bf16, name="cos")
        sinq = ropetab.tile([P, NQT, 512], bf16, name="sin")
        nc.gpsimd.dma_start(cosq[:], cosq_d.rearrange("(t p) c -> p t c", p=P))
        nc.gpsimd.dma_start(sinq[:], sinq_d.rearrange("(t p) c -> p t c", p=P))
        xq_sb = xpool.tile([P, DT, NQ], bf16, name="xsb")
        nc.gpsimd.dma_start(xq_sb[:], xq_d.rearrange("(t p) q -> p t q", p=P))
        for hc in range(HC):
            wqc = wstream.tile([P, DT, 512], bf16, name="wchunk")
            nc.gpsimd.dma_start(
                wqc[:],
                wq_d[:, hc * 512:(hc + 1) * 512].rearrange("(t p) o -> p t o", p=P))
            for qt in range(NQT):
                ps = pp.tile([P, 512], f32, name="pj")
                for dt in range(DT):
                    nc.tensor.matmul(ps[:], xq_sb[:, dt, qt * P:(qt + 1) * P],
                                     wqc[:, dt, :], start=(dt == 0),
                                     stop=(dt == DT - 1))
                rot = rope_block(ps, cosq, sinq, qt)
                for b in range(4):
                    pst = psc.tile([P, P], bf16, name="psS")
                    nc.tensor.transpose(pst[:], rot[:, b * P:(b + 1) * P], ident[:])
                    dst = Qt[:, hc * 4 + b, qt * P:(qt + 1) * P]
                    if b % 2 == 0:
                        nc.scalar.copy(dst, pst[:])
                    else:
                        nc.vector.tensor_copy(dst, pst[:])

        # ---- attention per tile t = heads (t, 16+t) ----
        rs_all = persist.tile([P, NQ], f32, name="rs_all")
        rs_rcp = persist.tile([P, NQ], bf16, name="rs_rcp")

        def norm_pass(trange, rows):
            with nc.allow_low_precision(reason="softmax denominator in bf16"):
                nc.vector.reciprocal(rs_rcp[rows], rs_all[rows])
            for t2 in trange:
                for par in range(2):
                    h_lo = par * 64
                    r = (t2 // 4) * 32 + (t2 % 4) * 2 + par
                    rcp0 = work.tile([1, NQ], bf16, name="rcp0")
                    nc.gpsimd.dma_start(rcp0[:], rs_rcp[r:r + 1, :])
                    for (qc, qcw) in qchunks:
                        psBt = pp.tile([P, 512], f32, name="pj")[0:64, :qcw]
                        nc.tensor.matmul(psBt, ones64[:], rcp0[:, qc:qc + qcw],
                                         start=True, stop=True)
                        sl = attnT[h_lo:h_lo + 64, t2, qc:qc + qcw]
                        nc.vector.tensor_mul(sl, sl, psBt)

        for t in range(HQT):
            groups = (t // 4, 4 + t // 4)
            for (qc, qcw) in qchunks:
                live = [kt for kt in range(NKT)
                        if kt * P <= qc + qcw - 1 + offs_max]
                psO = [pv.tile([P, 512], f32, name="pvo")[:, :qcw]
                       for _ in range(2)]
                pms = {}

                def qk_exp_mask(kt, par):
                    lo = max(0, kt * P - qc - offs_max)
                    g = groups[par]
                    psS = psc.tile([P, 512], f32, name="psS")[:, :qcw]
                    nc.tensor.matmul(
                        psS[:, lo:], KtRz[:, g, kt * P:(kt + 1) * P],
                        Qt[:, t, qc + lo:qc + qcw], start=True, stop=True)
                    if causal:
                        pm = pmpool.tile([P, 512], bf16, name="pm")[:, :qcw]
                        nc.scalar.activation(pm[:, lo:], psS[:, lo:], EXPF,
                                             bias=0.0, scale=0.125)
                        d0 = kt * P - qc
                        dlo, dhi = max(lo, d0), min(qcw, d0 + P)
                        if dlo < dhi:
                            nc.vector.tensor_mul(
                                pm[:, dlo:dhi], pm[:, dlo:dhi],
                                mask_sb[:, kt, qc + dlo:qc + dhi])
                    else:
                        pexp = pmpool.tile([P, 512], bf16, name="pexp")[:, :qcw]
                        nc.scalar.activation(pexp[:, lo:], psS[:, lo:], EXPF,
                                             bias=0.0, scale=0.125)
                        pm = pmpool.tile([P, 512], bf16, name="pm")[:, :qcw]
                        nc.vector.tensor_mul(pm[:, lo:], pexp[:, lo:],
                                             mask_sb[:, kt, qc + lo:qc + qcw])
                    return pm, lo

                def pv_mm(idx):
                    kt = live[idx]
                    for par in range(2):
                        pm, lo = pms[(idx, par)]
                        nc.tensor.matmul(
                            psO[par][:, lo:], Vaug[:, kt, groups[par], :],
                            pm[:, lo:], start=(idx == 0),
                            stop=(idx == len(live) - 1), skip_group_check=True)

                for idx, kt in enumerate(live):
                    for par in range(2):
                        pms[(idx, par)] = qk_exp_mask(kt, par)
                    if idx > 0:
                        pv_mm(idx - 1)
                        del pms[(idx - 1, 0)], pms[(idx - 1, 1)]
                pv_mm(len(live) - 1)

                for par in range(2):
                    h_lo = par * 64
                    dst = attnT[h_lo:h_lo + 64, t, qc:qc + qcw]
                    if par == 0:
                        nc.scalar.copy(dst, psO[par][0:64, :])
                    else:
                        nc.vector.tensor_copy(dst, psO[par][0:64, :])
                    rsum0 = work.tile([1, 512], f32, name="rsum0")[:, :qcw]
                    nc.vector.tensor_copy(rsum0, psO[par][64:65, :])
                    r = (t // 4) * 32 + (t % 4) * 2 + par
                    nc.gpsimd.dma_start(rs_all[r:r + 1, qc:qc + qcw], rsum0)
            if t % 4 == 3:
                qi = t // 4
                norm_pass(range(qi * 4, qi * 4 + 4), slice(qi * 32, qi * 32 + 8))

        # ---- output projection (wo rows head-permuted) ----
        for dc in range(4):
            woc = wstream.tile([P, DT, 512], bf16, name="wchunk")
            nc.gpsimd.dma_start(
                woc[:],
                wo_d[:, dc * 512:(dc + 1) * 512].rearrange("(t p) o -> p t o", p=P))
            for qt in range(NQT):
                ps = pp.tile([P, 512], f32, name="pj")
                for j in range(HQT):
                    nc.tensor.matmul(ps[:], attnT[:, j, qt * P:(qt + 1) * P],
                                     woc[:, j, :], start=(j == 0),
                                     stop=(j == HQT - 1))
                osb = work.tile([P, 512], f32, name="osb")
                nc.vector.tensor_copy(osb[:], ps[:])
                nc.gpsimd.dma_start(
                    out_d[qt * P:(qt + 1) * P, dc * 512:(dc + 1) * 512], osb[:])

    nc.finalize()
    _BUILD_CACHE[key] = nc
    return nc


# ---------------------------------------------------------------------------
# entry point
# ---------------------------------------------------------------------------

def kernel(x, freqs_cis, sequence_id, wq, wk, wv, wo):
    x = np.asarray(x, dtype=np.float32)
    freqs_cis = np.asarray(freqs_cis, dtype=np.float32)
    sequence_id = np.asarray(sequence_id)

    jobs = _plan_jobs(sequence_id)
    NQ = _round_up(max(max(j[2] for j in jobs), 1), P)
    NK = _round_up(max(max(j[1] + j[2] - j[3] for j in jobs), 1), P)
    offs_max = max(j[1] - j[3] for j in jobs)

    def single_doc(j):
        b, qs, ql, ks = j
        if ql == 0:
            return True
        seg = np.asarray(sequence_id[b])[ks:qs + ql]
        return bool((seg == seg[0]).all())

    causal = offs_max == 0 and all(single_doc(j) for j in jobs)

    cos_tab = freqs_cis[:, :, 0].astype(np.float32)
    sin_tab = freqs_cis[:, :, 1].astype(np.float32)
    wq_t = _permute_wq(np.ascontiguousarray(np.asarray(wq, np.float32).T)).astype(bf)
    wk_t = np.ascontiguousarray(np.asarray(wk, np.float32).T).astype(bf)
    wv_t = np.ascontiguousarray(np.asarray(wv, np.float32).T).astype(bf)
    wo_t = _permute_wo(np.ascontiguousarray(np.asarray(wo, np.float32).T)).astype(bf)
    id16 = np.eye(P, dtype=bf)

    in_maps = []
    for job in jobs:
        p = _core_inputs(job, NQ, NK, x, sequence_id, cos_tab, sin_tab)
        p.update({"wq_t": wq_t, "wk_t": wk_t, "wv_t": wv_t, "wo_t": wo_t,
                  "ident": id16})
        in_maps.append(p)

    nc = _build(NQ, NK, offs_max, causal)
    res = run_bass_kernel_spmd(nc, in_maps, core_ids=list(range(N_CORES)))

    full = np.zeros((BS, S, HQ), dtype=np.float32)
    for job, r in zip(jobs, res.results):
        b, qs, ql, ks = job
        if ql > 0:
            full[b, qs:qs + ql] = r["out"][:ql]
    return full


# revision 2
# speedup vs baseline: 1.0741x; 1.0741x over previous
"""Document-causal GQA attention on 8 TRN2 NeuronCores.

Strategy: the packed-document mask makes attention block-diagonal over
(batch, document) segments, so each of the 8 cores gets one segment's
queries (2 batches x ~4 docs) together with its KV window — no
cross-core communication at all. The host shards/transposes inputs,
each core runs the full QKV->RoPE->softmax->PV->Wo pipeline on its
rows, and the host scatters the disjoint output rows back.

Device kernel (SPMD, one graph): bf16 matmuls (FWL-eligible 128x128
stationary operands via a head permutation + zero-padded K/V weights),
fp32 PSUM, ACT exp with folded 1/sqrt(hd) scale, diagonal-block-only
masking in the pure-causal case, softmax denominators via a ones
column appended to V, batched reciprocal + ones-outer-product
broadcast for normalization.
"""
import numpy as np
import ml_dtypes

from contextlib import ExitStack

import concourse.bass as bass
import concourse.tile as tile
from concourse import bacc, mybir
from concourse.bass_utils import run_bass_kernel_spmd

BS, S, D, H, KVH, HD = 2, 2048, 2048, 32, 8, 64
N_REP = H // KVH
HQ = H * HD
HKV = KVH * HD
P = 128
N_CORES = 8
DT = D // P
HC = HQ // 512
HQT = HQ // P

f32 = mybir.dt.float32
bf16 = mybir.dt.bfloat16
EXPF = mybir.ActivationFunctionType.Exp
bf = ml_dtypes.bfloat16

HEAD_ORDER = [i // 2 if i % 2 == 0 else 16 + i // 2 for i in range(32)]


# ---------------------------------------------------------------------------
# host-side planning
# ---------------------------------------------------------------------------

def _round_up(x, m):
    return ((x + m - 1) // m) * m


def _plan_jobs(sequence_id):
    jobs = []
    for b in range(BS):
        sid = np.asarray(sequence_id[b])
        starts = [0] + list(np.where(np.diff(sid) != 0)[0] + 1) + [len(sid)]
        for i in range(len(starts) - 1):
            jobs.append([b, int(starts[i]), int(starts[i + 1] - starts[i]),
                         int(starts[i])])
    while len(jobs) > N_CORES:
        best, bi = None, -1
        for i in range(len(jobs) - 1):
            a, c = jobs[i], jobs[i + 1]
            if a[0] == c[0] and a[1] + a[2] == c[1]:
                cost = (c[1] + c[2]) - min(a[3], c[3])
                if best is None or cost < best:
                    best, bi = cost, i
        a, c = jobs[bi], jobs[bi + 1]
        jobs[bi] = [a[0], a[1], a[2] + c[2], min(a[3], c[3])]
        del jobs[bi + 1]
    while len(jobs) < N_CORES:
        i = max(range(len(jobs)), key=lambda j: jobs[j][2])
        b, qs, ql, ks = jobs[i]
        if ql < 2:
            jobs.append([b, qs, 0, qs])
            continue
        h = ql // 2
        jobs[i] = [b, qs, h, ks]
        jobs.insert(i + 1, [b, qs + h, ql - h, ks])
    return jobs


def _permute_wq(wq_t):
    return np.ascontiguousarray(
        wq_t.reshape(D, 32, 64)[:, HEAD_ORDER, :].reshape(D, HQ))


def _permute_wo(wo_t):
    return np.ascontiguousarray(
        wo_t.reshape(32, 64, D)[HEAD_ORDER].reshape(HQ, D))


def _core_inputs(job, NQ, NK, x, sequence_id, cos_tab, sin_tab):
    b, qs, ql, ks = job
    kl = qs + ql - ks

    xq_t = np.zeros((D, NQ), dtype=np.float32)
    xq_t[:, :ql] = x[b, qs:qs + ql].T
    xk_t = np.zeros((D, NK), dtype=np.float32)
    xk_t[:, :kl] = x[b, ks:ks + kl].T

    def rope(start, ln, n):
        cos = np.ones((n, 64), dtype=np.float32)
        sin = np.zeros((n, 64), dtype=np.float32)
        c = cos_tab[start:start + ln]
        s = sin_tab[start:start + ln]
        cos[:ln, 0::2] = c
        cos[:ln, 1::2] = c
        sin[:ln, 0::2] = -s
        sin[:ln, 1::2] = s
        return (np.tile(cos, (1, 8)).astype(bf), np.tile(sin, (1, 8)).astype(bf))

    cos_q, sin_q = rope(qs, ql, NQ)
    cos_k, sin_k = rope(ks, kl, NK)

    sid = np.asarray(sequence_id[b])
    sid_q = np.full(NQ, -2, dtype=np.int64)
    sid_q[:ql] = sid[qs:qs + ql]
    sid_k = np.full(NK, -1, dtype=np.int64)
    sid_k[:kl] = sid[ks:ks + kl]
    gq = qs + np.arange(NQ)
    gk = ks + np.arange(NK)
    mask = ((sid_k[:, None] == sid_q[None, :]) &
            (gk[:, None] <= gq[None, :])).astype(np.float32)
    # padded query columns attend to key 0 so denominators stay finite
    mask[0, ql:] = 1.0
    kones = np.zeros((NK, 1), dtype=np.float32)
    kones[:kl] = 1.0

    return {
        "xq_t": xq_t.astype(bf), "xk_t": xk_t.astype(bf),
        "cos_q": cos_q, "sin_q": sin_q, "cos_k": cos_k, "sin_k": sin_k,
        "maskm": mask.astype(bf), "kones": kones.astype(bf),
    }


# ---------------------------------------------------------------------------
# device graph
# ---------------------------------------------------------------------------

_BUILD_CACHE = {}


def _build(NQ, NK, offs_max, causal):
    key = (NQ, NK, offs_max, causal)
    if key in _BUILD_CACHE:
        return _BUILD_CACHE[key]
    NQT, NKT = NQ // P, NK // P
    qchunks = [(c * 512, min(512, NQ - c * 512)) for c in range((NQ + 511) // 512)]

    nc = bacc.Bacc("TRN2", target_bir_lowering=False, debug=False,
                   num_devices=N_CORES)

    xq_d = nc.dram_tensor("xq_t", [D, NQ], bf16, kind="ExternalInput").ap()
    xk_d = nc.dram_tensor("xk_t", [D, NK], bf16, kind="ExternalInput").ap()
    wq_d = nc.dram_tensor("wq_t", [D, HQ], bf16, kind="ExternalInput").ap()
    wk_d = nc.dram_tensor("wk_t", [D, HKV], bf16, kind="ExternalInput").ap()
    wv_d = nc.dram_tensor("wv_t", [D, HKV], bf16, kind="ExternalInput").ap()
    wo_d = nc.dram_tensor("wo_t", [HQ, D], bf16, kind="ExternalInput").ap()
    cosq_d = nc.dram_tensor("cos_q", [NQ, 512], bf16, kind="ExternalInput").ap()
    sinq_d = nc.dram_tensor("sin_q", [NQ, 512], bf16, kind="ExternalInput").ap()
    cosk_d = nc.dram_tensor("cos_k", [NK, 512], bf16, kind="ExternalInput").ap()
    sink_d = nc.dram_tensor("sin_k", [NK, 512], bf16, kind="ExternalInput").ap()
    mask_d = nc.dram_tensor("maskm", [NK, NQ], bf16, kind="ExternalInput").ap()
    kones_d = nc.dram_tensor("kones", [NK, 1], bf16, kind="ExternalInput").ap()
    id_d = nc.dram_tensor("ident", [P, P], bf16, kind="ExternalInput").ap()
    out_d = nc.dram_tensor("out", [NQ, HQ], f32, kind="ExternalOutput").ap()

    with tile.TileContext(nc) as tc, ExitStack() as ctx:
        const = ctx.enter_context(tc.tile_pool(name="const", bufs=1))
        persist = ctx.enter_context(tc.tile_pool(name="persist", bufs=1))
        xpool = ctx.enter_context(tc.tile_pool(name="xpool", bufs=2))
        wstream = ctx.enter_context(tc.tile_pool(name="wstream", bufs=2))
        work = ctx.enter_context(tc.tile_pool(name="work", bufs=2))
        ropetab = ctx.enter_context(tc.tile_pool(name="ropetab", bufs=1))
        pmpool = ctx.enter_context(tc.tile_pool(name="pmpool", bufs=10))
        pp = ctx.enter_context(tc.tile_pool(name="pp", bufs=2, space="PSUM"))
        psc = ctx.enter_context(tc.tile_pool(name="psc", bufs=3, space="PSUM"))
        pv = ctx.enter_context(tc.tile_pool(name="pv", bufs=3, space="PSUM"))

        # ---- initial loads: x/w chunks first so the PE starts ASAP ----
        xk_sb = xpool.tile([P, DT, NK], bf16, name="xsb")
        xk_r = xk_d.rearrange("(t p) q -> p t q", p=P)
        wkc = wstream.tile([P, DT, 512], bf16, name="wchunk")
        wk_r = wk_d.rearrange("(t p) o -> p t o", p=P)
        for a, b2 in [(0, 1), (1, 2), (2, 4), (4, 8), (8, 16)]:
            nc.sync.dma_start(xk_sb[:, a:b2, :], xk_r[:, a:b2, :])
            nc.sync.dma_start(wkc[:, a:b2, :], wk_r[:, a:b2, :])

        ident = const.tile([P, P], bf16, name="ident")
        nc.sync.dma_start(ident[:], id_d)
        ones64 = const.tile([1, HD], bf16, name="ones64")
        nc.vector.memset(ones64[:], 1.0)

        Qt = persist.tile([P, HQT, NQ], bf16, name="Qt")
        KtRz = persist.tile([P, KVH, NK], bf16, name="KtRz")
        Vaug = persist.tile([P, NKT, KVH, P], bf16, name="Vaug")
        attnT = persist.tile([P, HQT, NQ], bf16, name="attnT")
        mask_sb = persist.tile([P, NKT, NQ], bf16, name="mask_sb")

        nc.vector.memset(KtRz[64:128, 0:4, :], 0.0)
        nc.vector.memset(KtRz[0:64, 4:8, :], 0.0)
        nc.vector.memset(Vaug[:, :, :, HD:P], 0.0)
        kones_sb = const.tile([P, NKT], bf16, name="kones_sb")
        nc.sync.dma_start(kones_sb[:], kones_d.rearrange("(t p) o -> p (t o)", p=P))
        for kt in range(NKT):
            for g in range(KVH):
                nc.vector.tensor_copy(Vaug[:, kt, g, HD:HD + 1],
                                      kones_sb[:, kt:kt + 1])

        cosk = ropetab.tile([P, NKT, 512], bf16, name="cos")
        sink = ropetab.tile([P, NKT, 512], bf16, name="sin")
        nc.sync.dma_start(cosk[:], cosk_d.rearrange("(t p) c -> p t c", p=P))
        nc.sync.dma_start(sink[:], sink_d.rearrange("(t p) c -> p t c", p=P))
        nc.sync.dma_start(mask_sb[:], mask_d.rearrange("(t p) q -> p t q", p=P))

        def rope_block(ps, cos_t, sin_t, ti):
            nat = work.tile([P, 512], f32, name="nat")
            nc.vector.tensor_copy(nat[:], ps[:])
            ro = work.tile([P, 512], f32, name="ro")
            nc.gpsimd.tensor_mul(ro[:, 0::2], nat[:, 1::2], sin_t[:, ti, 0::2])
            nc.gpsimd.tensor_mul(ro[:, 1::2], nat[:, 0::2], sin_t[:, ti, 1::2])
            tmp = work.tile([P, 512], f32, name="tmp")
            nc.vector.tensor_mul(tmp[:], nat[:], cos_t[:, ti, :])
            rot = work.tile([P, 512], bf16, name="rot")
            nc.vector.tensor_add(rot[:], ro[:], tmp[:])
            return rot

        # ---- K projection + rope + transpose (zero-padded halves) ----
        for kt in range(NKT):
            ps = pp.tile([P, 512], f32, name="pj")
            for dt in range(DT):
                nc.tensor.matmul(ps[:], xk_sb[:, dt, kt * P:(kt + 1) * P],
                                 wkc[:, dt, :], start=(dt == 0),
                                 stop=(dt == DT - 1))
            rot = rope_block(ps, cosk, sink, kt)
            ks = slice(kt * P, (kt + 1) * P)
            for b in range(4):
                pst = psc.tile([P, P], bf16, name="psS")
                nc.tensor.transpose(pst[:], rot[:, b * P:(b + 1) * P], ident[:])
                half = (2 * b) // 4
                lo = half * 64
                nc.scalar.copy(KtRz[lo:lo + 64, 2 * b, ks], pst[0:64, :])
                nc.scalar.copy(KtRz[lo:lo + 64, 2 * b + 1, ks], pst[64:128, :])

        # ---- V projection -> Vaug ----
        wvc = wstream.tile([P, DT, 512], bf16, name="wchunk")
        nc.sync.dma_start(wvc[:], wv_d.rearrange("(t p) o -> p t o", p=P))
        for kt in range(NKT):
            ps = pp.tile([P, 512], f32, name="pj")
            for dt in range(DT):
                nc.tensor.matmul(ps[:], xk_sb[:, dt, kt * P:(kt + 1) * P],
                                 wvc[:, dt, :], start=(dt == 0),
                                 stop=(dt == DT - 1))
            nc.vector.tensor_copy(Vaug[:, kt, :, 0:HD],
                                  ps[:].rearrange("p (g d) -> p g d", g=KVH))

        # ---- Q projection + rope + transpose (head-permuted wq) ----
        cosq = ropetab.tile([P, NQT, 512], bf16, name="cos")
        sinq = ropetab.tile([P, NQT, 512], bf16, name="sin")
        nc.sync.dma_start(cosq[:], cosq_d.rearrange("(t p) c -> p t c", p=P))
        nc.sync.dma_start(sinq[:], sinq_d.rearrange("(t p) c -> p t c", p=P))
        xq_sb = xpool.tile([P, DT, NQ], bf16, name="xsb")
        nc.sync.dma_start(xq_sb[:], xq_d.rearrange("(t p) q -> p t q", p=P))
        for hc in range(HC):
            wqc = wstream.tile([P, DT, 512], bf16, name="wchunk")
            nc.sync.dma_start(
                wqc[:],
                wq_d[:, hc * 512:(hc + 1) * 512].rearrange("(t p) o -> p t o", p=P))
            for qt in range(NQT):
                ps = pp.tile([P, 512], f32, name="pj")
                for dt in range(DT):
                    nc.tensor.matmul(ps[:], xq_sb[:, dt, qt * P:(qt + 1) * P],
                                     wqc[:, dt, :], start=(dt == 0),
                                     stop=(dt == DT - 1))
                rot = rope_block(ps, cosq, sinq, qt)
                for b in range(4):
                    pst = psc.tile([P, P], bf16, name="psS")
                    nc.tensor.transpose(pst[:], rot[:, b * P:(b + 1) * P], ident[:])
                    dst = Qt[:, hc * 4 + b, qt * P:(qt + 1) * P]
                    if b % 2 == 0:
                        nc.scalar.copy(dst, pst[:])
                    else:
                        nc.vector.tensor_copy(dst, pst[:])

        # ---- attention per tile t = heads (t, 16+t) ----
        rs_all = persist.tile([P, NQ], f32, name="rs_all")
        rs_rcp = persist.tile([P, NQ], bf16, name="rs_rcp")

        def norm_pass(trange, rows):
            with nc.allow_low_precision(reason="softmax denominator in bf16"):
                nc.vector.reciprocal(rs_rcp[rows], rs_all[rows])
            for t2 in trange:
                for par in range(2):
                    h_lo = par * 64
                    r = (t2 // 4) * 32 + (t2 % 4) * 2 + par
                    rcp0 = work.tile([1, NQ], bf16, name="rcp0")
                    nc.sync.dma_start(rcp0[:], rs_rcp[r:r + 1, :])
                    for (qc, qcw) in qchunks:
                        psBt = pp.tile([P, 512], f32, name="pj")[0:64, :qcw]
                        nc.tensor.matmul(psBt, ones64[:], rcp0[:, qc:qc + qcw],
                                         start=True, stop=True)
                        sl = attnT[h_lo:h_lo + 64, t2, qc:qc + qcw]
                        nc.vector.tensor_mul(sl, sl, psBt)

        for t in range(HQT):
            groups = (t // 4, 4 + t // 4)
            for (qc, qcw) in qchunks:
                live = [kt for kt in range(NKT)
                        if kt * P <= qc + qcw - 1 + offs_max]
                psO = [pv.tile([P, 512], f32, name="pvo")[:, :qcw]
                       for _ in range(2)]
                pms = {}

                def qk_exp_mask(kt, par):
                    lo = max(0, kt * P - qc - offs_max)
                    g = groups[par]
                    psS = psc.tile([P, 512], f32, name="psS")[:, :qcw]
                    nc.tensor.matmul(
                        psS[:, lo:], KtRz[:, g, kt * P:(kt + 1) * P],
                        Qt[:, t, qc + lo:qc + qcw], start=True, stop=True)
                    if causal:
                        pm = pmpool.tile([P, 512], bf16, name="pm")[:, :qcw]
                        nc.scalar.activation(pm[:, lo:], psS[:, lo:], EXPF,
                                             bias=0.0, scale=0.125)
                        d0 = kt * P - qc
                        dlo, dhi = max(lo, d0), min(qcw, d0 + P)
                        if dlo < dhi:
                            nc.vector.tensor_mul(
                                pm[:, dlo:dhi], pm[:, dlo:dhi],
                                mask_sb[:, kt, qc + dlo:qc + dhi])
                    else:
                        pexp = pmpool.tile([P, 512], bf16, name="pexp")[:, :qcw]
                        nc.scalar.activation(pexp[:, lo:], psS[:, lo:], EXPF,
                                             bias=0.0, scale=0.125)
                        pm = pmpool.tile([P, 512], bf16, name="pm")[:, :qcw]
                        nc.vector.tensor_mul(pm[:, lo:], pexp[:, lo:],
                                             mask_sb[:, kt, qc + lo:qc + qcw])
                    return pm, lo

                def pv_mm(idx):
                    kt = live[idx]
                    for par in range(2):
                        pm, lo = pms[(idx, par)]
                        nc.tensor.matmul(
                            psO[par][:, lo:], Vaug[:, kt, groups[par], :],
                            pm[:, lo:], start=(idx == 0),
                            stop=(idx == len(live) - 1), skip_group_check=True)

                for idx, kt in enumerate(live):
                    for par in range(2):
                        pms[(idx, par)] = qk_exp_mask(kt, par)
                    if idx > 0:
                        pv_mm(idx - 1)
                        del pms[(idx - 1, 0)], pms[(idx - 1, 1)]
                pv_mm(len(live) - 1)

                for par in range(2):
                    h_lo = par * 64
                    dst = attnT[h_lo:h_lo + 64, t, qc:qc + qcw]
                    if par == 0:
                        nc.scalar.copy(dst, psO[par][0:64, :])
                    else:
                        nc.vector.tensor_copy(dst, psO[par][0:64, :])
                    rsum0 = work.tile([1, 512], f32, name="rsum0")[:, :qcw]
                    nc.vector.tensor_copy(rsum0, psO[par][64:65, :])
                    r = (t // 4) * 32 + (t % 4) * 2 + par
                    nc.sync.dma_start(rs_all[r:r + 1, qc:qc + qcw], rsum0)
            if t % 4 == 3:
                qi = t // 4
                norm_pass(range(qi * 4, qi * 4 + 4), slice(qi * 32, qi * 32 + 8))

        # ---- output projection (wo rows head-permuted) ----
        for dc in range(4):
            woc = wstream.tile([P, DT, 512], bf16, name="wchunk")
            nc.sync.dma_start(
                woc[:],
                wo_d[:, dc * 512:(dc + 1) * 512].rearrange("(t p) o -> p t o", p=P))
            for qt in range(NQT):
                ps = pp.tile([P, 512], f32, name="pj")
                for j in range(HQT):
                    nc.tensor.matmul(ps[:], attnT[:, j, qt * P:(qt + 1) * P],
                                     woc[:, j, :], start=(j == 0),
                                     stop=(j == HQT - 1))
                osb = work.tile([P, 512], f32, name="osb")
                nc.vector.tensor_copy(osb[:], ps[:])
                nc.sync.dma_start(
                    out_d[qt * P:(qt + 1) * P, dc * 512:(dc + 1) * 512], osb[:])

    nc.finalize()
    _BUILD_CACHE[key] = nc
    return nc


# ---------------------------------------------------------------------------
# entry point
# ---------------------------------------------------------------------------

def kernel(x, freqs_cis, sequence_id, wq, wk, wv, wo):
    x = np.asarray(x, dtype=np.float32)
    freqs_cis = np.asarray(freqs_cis, dtype=np.float32)
    sequence_id = np.asarray(sequence_id)

    jobs = _plan_jobs(sequence_id)
    NQ = _round_up(max(max(j[2] for j in jobs), 1), P)
    NK = _round_up(max(max(j[1] + j[2] - j[3] for j in jobs), 1), P)
    offs_max = max(j[1] - j[3] for j in jobs)

    def single_doc(j):
        b, qs, ql, ks = j
        if ql == 0:
            return True
        seg = np.asarray(sequence_id[b])[ks:qs + ql]
        return bool((seg == seg[0]).all())

    causal = offs_max == 0 and all(single_doc(j) for j in jobs)

    cos_tab = freqs_cis[:, :, 0].astype(np.float32)
    sin_tab = freqs_cis[:, :, 1].astype(np.float32)
    wq_t = _permute_wq(np.ascontiguousarray(np.asarray(wq, np.float32).T)).astype(bf)
    wk_t = np.ascontiguousarray(np.asarray(wk, np.float32).T).astype(bf)
    wv_t = np.ascontiguousarray(np.asarray(wv, np.float32).T).astype(bf)
    wo_t = _permute_wo(np.ascontiguousarray(np.asarray(wo, np.float32).T)).astype(bf)
    id16 = np.eye(P, dtype=bf)

    in_maps = []
    for job in jobs:
        p = _core_inputs(job, NQ, NK, x, sequence_id, cos_tab, sin_tab)
        p.update({"wq_t": wq_t, "wk_t": wk_t, "wv_t": wv_t, "wo_t": wo_t,
                  "ident": id16})
        in_maps.append(p)

    nc = _build(NQ, NK, offs_max, causal)
    res = run_bass_kernel_spmd(nc, in_maps, core_ids=list(range(N_CORES)))

    full = np.zeros((BS, S, HQ), dtype=np.float32)
    for job, r in zip(jobs, res.results):
        b, qs, ql, ks = job
        if ql > 0:
            full[b, qs:qs + ql] = r["out"][:ql]
    return full


# revision 4
# speedup vs baseline: 1.1241x; 1.0466x over previous
"""Document-causal GQA attention on 8 TRN2 NeuronCores.

Strategy: the packed-document mask makes attention block-diagonal over
(batch, document) segments, so each of the 8 cores gets one segment's
queries (2 batches x ~4 docs) together with its KV window — no
cross-core communication at all. The host shards/transposes inputs,
each core runs the full QKV->RoPE->softmax->PV->Wo pipeline on its
rows, and the host scatters the disjoint output rows back.

Device kernel (SPMD, one graph): bf16 matmuls (FWL-eligible 128x128
stationary operands via a head permutation + zero-padded K/V weights),
fp32 PSUM, ACT exp with folded 1/sqrt(hd) scale, diagonal-block-only
masking in the pure-causal case, softmax denominators via a ones
column appended to V, batched reciprocal + ones-outer-product
broadcast for normalization.
"""
import numpy as np
import ml_dtypes

from contextlib import ExitStack

import concourse.bass as bass
import concourse.tile as tile
from concourse import bacc, mybir
from concourse.bass_utils import run_bass_kernel_spmd

BS, S, D, H, KVH, HD = 2, 2048, 2048, 32, 8, 64
N_REP = H // KVH
HQ = H * HD
HKV = KVH * HD
P = 128
N_CORES = 8
DT = D // P
HC = HQ // 512
HQT = HQ // P

f32 = mybir.dt.float32
bf16 = mybir.dt.bfloat16
EXPF = mybir.ActivationFunctionType.Exp
bf = ml_dtypes.bfloat16

HEAD_ORDER = [i // 2 if i % 2 == 0 else 16 + i // 2 for i in range(32)]


# ---------------------------------------------------------------------------
# host-side planning
# ---------------------------------------------------------------------------

def _round_up(x, m):
    return ((x + m - 1) // m) * m


def _plan_jobs(sequence_id):
    jobs = []
    for b in range(BS):
        sid = np.asarray(sequence_id[b])
        starts = [0] + list(np.where(np.diff(sid) != 0)[0] + 1) + [len(sid)]
        for i in range(len(starts) - 1):
            jobs.append([b, int(starts[i]), int(starts[i + 1] - starts[i]),
                         int(starts[i])])
    while len(jobs) > N_CORES:
        best, bi = None, -1
        for i in range(len(jobs) - 1):
            a, c = jobs[i], jobs[i + 1]
            if a[0] == c[0] and a[1] + a[2] == c[1]:
                cost = (c[1] + c[2]) - min(a[3], c[3])
                if best is None or cost < best:
                    best, bi = cost, i
        a, c = jobs[bi], jobs[bi + 1]
        jobs[bi] = [a[0], a[1], a[2] + c[2], min(a[3], c[3])]
        del jobs[bi + 1]
    while len(jobs) < N_CORES:
        i = max(range(len(jobs)), key=lambda j: jobs[j][2])
        b, qs, ql, ks = jobs[i]
        if ql < 2:
            jobs.append([b, qs, 0, qs])
            continue
        h = ql // 2
        jobs[i] = [b, qs, h, ks]
        jobs.insert(i + 1, [b, qs + h, ql - h, ks])
    return jobs


def _permute_wq(wq_t):
    return np.ascontiguousarray(
        wq_t.reshape(D, 32, 64)[:, HEAD_ORDER, :].reshape(D, HQ))


def _permute_wo(wo_t):
    return np.ascontiguousarray(
        wo_t.reshape(32, 64, D)[HEAD_ORDER].reshape(HQ, D))


def _core_inputs(job, NQ, NK, x, sequence_id, cos_tab, sin_tab):
    b, qs, ql, ks = job
    kl = qs + ql - ks

    xq_t = np.zeros((D, NQ), dtype=np.float32)
    xq_t[:, :ql] = x[b, qs:qs + ql].T
    xk_t = np.zeros((D, NK), dtype=np.float32)
    xk_t[:, :kl] = x[b, ks:ks + kl].T

    def rope(start, ln, n):
        cos = np.ones((n, 64), dtype=np.float32)
        sin = np.zeros((n, 64), dtype=np.float32)
        c = cos_tab[start:start + ln]
        s = sin_tab[start:start + ln]
        cos[:ln, 0::2] = c
        cos[:ln, 1::2] = c
        sin[:ln, 0::2] = -s
        sin[:ln, 1::2] = s
        return (np.tile(cos, (1, 8)).astype(bf), np.tile(sin, (1, 8)).astype(bf))

    cos_q, sin_q = rope(qs, ql, NQ)
    cos_k, sin_k = rope(ks, kl, NK)

    sid = np.asarray(sequence_id[b])
    sid_q = np.full(NQ, -2, dtype=np.int64)
    sid_q[:ql] = sid[qs:qs + ql]
    sid_k = np.full(NK, -1, dtype=np.int64)
    sid_k[:kl] = sid[ks:ks + kl]
    gq = qs + np.arange(NQ)
    gk = ks + np.arange(NK)
    mask = ((sid_k[:, None] == sid_q[None, :]) &
            (gk[:, None] <= gq[None, :])).astype(np.float32)
    # padded query columns attend to key 0 so denominators stay finite
    mask[0, ql:] = 1.0
    kones = np.zeros((NK, 1), dtype=np.float32)
    kones[:kl] = 1.0

    return {
        "xq_t": xq_t.astype(bf), "xk_t": xk_t.astype(bf),
        "cos_q": cos_q, "sin_q": sin_q, "cos_k": cos_k, "sin_k": sin_k,
        "maskm": mask.astype(bf), "kones": kones.astype(bf),
    }


# ---------------------------------------------------------------------------
# device graph
# ---------------------------------------------------------------------------

_BUILD_CACHE = {}


def _build(NQ, NK, offs_max, causal):
    key = (NQ, NK, offs_max, causal)
    if key in _BUILD_CACHE:
        return _BUILD_CACHE[key]
    NQT, NKT = NQ // P, NK // P
    qchunks = [(c * 512, min(512, NQ - c * 512)) for c in range((NQ + 511) // 512)]

    nc = bacc.Bacc("TRN2", target_bir_lowering=False, debug=False,
                   num_devices=N_CORES)

    xq_d = nc.dram_tensor("xq_t", [D, NQ], bf16, kind="ExternalInput").ap()
    xk_d = nc.dram_tensor("xk_t", [D, NK], bf16, kind="ExternalInput").ap()
    wq_d = nc.dram_tensor("wq_t", [D, HQ], bf16, kind="ExternalInput").ap()
    wk_d = nc.dram_tensor("wk_t", [D, HKV], bf16, kind="ExternalInput").ap()
    wv_d = nc.dram_tensor("wv_t", [D, HKV], bf16, kind="ExternalInput").ap()
    wo_d = nc.dram_tensor("wo_t", [HQ, D], bf16, kind="ExternalInput").ap()
    cosq_d = nc.dram_tensor("cos_q", [NQ, 512], bf16, kind="ExternalInput").ap()
    sinq_d = nc.dram_tensor("sin_q", [NQ, 512], bf16, kind="ExternalInput").ap()
    cosk_d = nc.dram_tensor("cos_k", [NK, 512], bf16, kind="ExternalInput").ap()
    sink_d = nc.dram_tensor("sin_k", [NK, 512], bf16, kind="ExternalInput").ap()
    mask_d = nc.dram_tensor("maskm", [NK, NQ], bf16, kind="ExternalInput").ap()
    kones_d = nc.dram_tensor("kones", [NK, 1], bf16, kind="ExternalInput").ap()
    id_d = nc.dram_tensor("ident", [P, P], bf16, kind="ExternalInput").ap()
    out_d = nc.dram_tensor("out", [NQ, HQ], f32, kind="ExternalOutput").ap()

    with tile.TileContext(nc) as tc, ExitStack() as ctx:
        const = ctx.enter_context(tc.tile_pool(name="const", bufs=1))
        persist = ctx.enter_context(tc.tile_pool(name="persist", bufs=1))
        xpool = ctx.enter_context(tc.tile_pool(name="xpool", bufs=2))
        wstream = ctx.enter_context(tc.tile_pool(name="wstream", bufs=2))
        work = ctx.enter_context(tc.tile_pool(name="work", bufs=2))
        ropetab = ctx.enter_context(tc.tile_pool(name="ropetab", bufs=1))
        pmpool = ctx.enter_context(tc.tile_pool(name="pmpool", bufs=10))
        pp = ctx.enter_context(tc.tile_pool(name="pp", bufs=2, space="PSUM"))
        psc = ctx.enter_context(tc.tile_pool(name="psc", bufs=3, space="PSUM"))
        pv = ctx.enter_context(tc.tile_pool(name="pv", bufs=3, space="PSUM"))

        # ---- initial loads: x/w chunks first so the PE starts ASAP ----
        xk_sb = xpool.tile([P, DT, NK], bf16, name="xsb")
        xk_r = xk_d.rearrange("(t p) q -> p t q", p=P)
        wkc = wstream.tile([P, DT, 512], bf16, name="wchunk")
        wk_r = wk_d.rearrange("(t p) o -> p t o", p=P)
        for a, b2 in [(0, 1), (1, 2), (2, 4), (4, 8), (8, 16)]:
            nc.sync.dma_start(xk_sb[:, a:b2, :], xk_r[:, a:b2, :])
            nc.sync.dma_start(wkc[:, a:b2, :], wk_r[:, a:b2, :])

        ident = const.tile([P, P], bf16, name="ident")
        nc.sync.dma_start(ident[:], id_d)
        ones64 = const.tile([1, HD], bf16, name="ones64")
        nc.vector.memset(ones64[:], 1.0)

        Qt = persist.tile([P, HQT, NQ], bf16, name="Qt")
        KtRz = persist.tile([P, KVH, NK], bf16, name="KtRz")
        Vaug = persist.tile([P, NKT, KVH, P], bf16, name="Vaug")
        attnT = persist.tile([P, HQT, NQ], bf16, name="attnT")
        mask_sb = persist.tile([P, NKT, NQ], bf16, name="mask_sb")

        nc.vector.memset(KtRz[64:128, 0:4, :], 0.0)
        nc.vector.memset(KtRz[0:64, 4:8, :], 0.0)
        nc.vector.memset(Vaug[:, :, :, HD:P], 0.0)
        kones_sb = const.tile([P, NKT], bf16, name="kones_sb")
        nc.sync.dma_start(kones_sb[:], kones_d.rearrange("(t p) o -> p (t o)", p=P))
        for kt in range(NKT):
            for g in range(KVH):
                nc.vector.tensor_copy(Vaug[:, kt, g, HD:HD + 1],
                                      kones_sb[:, kt:kt + 1])

        cosk = ropetab.tile([P, NKT, 512], bf16, name="cos")
        sink = ropetab.tile([P, NKT, 512], bf16, name="sin")
        nc.sync.dma_start(cosk[:], cosk_d.rearrange("(t p) c -> p t c", p=P))
        nc.sync.dma_start(sink[:], sink_d.rearrange("(t p) c -> p t c", p=P))
        nc.sync.dma_start(mask_sb[:], mask_d.rearrange("(t p) q -> p t q", p=P))

        def rope_block(ps, cos_t, sin_t, ti):
            nat = work.tile([P, 512], f32, name="nat")
            nc.vector.tensor_copy(nat[:], ps[:])
            ro = work.tile([P, 512], f32, name="ro")
            nc.gpsimd.tensor_mul(ro[:, 0::2], nat[:, 1::2], sin_t[:, ti, 0::2])
            nc.gpsimd.tensor_mul(ro[:, 1::2], nat[:, 0::2], sin_t[:, ti, 1::2])
            tmp = work.tile([P, 512], f32, name="tmp")
            nc.vector.tensor_mul(tmp[:], nat[:], cos_t[:, ti, :])
            rot = work.tile([P, 512], bf16, name="rot")
            nc.vector.tensor_add(rot[:], ro[:], tmp[:])
            return rot

        # ---- K projection + rope + transpose (zero-padded halves) ----
        # rope+transpose evictions run one tile behind the projection
        # matmuls so the PE never stalls on the rope chain
        def k_evict(ps, kt):
            rot = rope_block(ps, cosk, sink, kt)
            ks = slice(kt * P, (kt + 1) * P)
            for b in range(4):
                pst = psc.tile([P, P], bf16, name="psS")
                nc.tensor.transpose(pst[:], rot[:, b * P:(b + 1) * P], ident[:])
                half = (2 * b) // 4
                lo = half * 64
                nc.scalar.copy(KtRz[lo:lo + 64, 2 * b, ks], pst[0:64, :])
                nc.scalar.copy(KtRz[lo:lo + 64, 2 * b + 1, ks], pst[64:128, :])

        prevk = None
        for kt in range(NKT):
            ps = pp.tile([P, 512], f32, name="pj")
            for dt in range(DT):
                nc.tensor.matmul(ps[:], xk_sb[:, dt, kt * P:(kt + 1) * P],
                                 wkc[:, dt, :], start=(dt == 0),
                                 stop=(dt == DT - 1))
            if prevk is not None:
                k_evict(*prevk)
            prevk = (ps, kt)
        k_evict(*prevk)

        # ---- V projection -> Vaug ----
        wvc = wstream.tile([P, DT, 512], bf16, name="wchunk")
        nc.sync.dma_start(wvc[:], wv_d.rearrange("(t p) o -> p t o", p=P))
        for kt in range(NKT):
            ps = pp.tile([P, 512], f32, name="pj")
            for dt in range(DT):
                nc.tensor.matmul(ps[:], xk_sb[:, dt, kt * P:(kt + 1) * P],
                                 wvc[:, dt, :], start=(dt == 0),
                                 stop=(dt == DT - 1))
            nc.vector.tensor_copy(Vaug[:, kt, :, 0:HD],
                                  ps[:].rearrange("p (g d) -> p g d", g=KVH))

        # ---- Q projection + rope + transpose (head-permuted wq) ----
        cosq = ropetab.tile([P, NQT, 512], bf16, name="cos")
        sinq = ropetab.tile([P, NQT, 512], bf16, name="sin")
        nc.sync.dma_start(cosq[:], cosq_d.rearrange("(t p) c -> p t c", p=P))
        nc.sync.dma_start(sinq[:], sinq_d.rearrange("(t p) c -> p t c", p=P))
        xq_sb = xpool.tile([P, DT, NQ], bf16, name="xsb")
        nc.sync.dma_start(xq_sb[:], xq_d.rearrange("(t p) q -> p t q", p=P))
        def q_evict(ps, hc, qt):
            rot = rope_block(ps, cosq, sinq, qt)
            for b in range(4):
                pst = psc.tile([P, P], bf16, name="psS")
                nc.tensor.transpose(pst[:], rot[:, b * P:(b + 1) * P], ident[:])
                dst = Qt[:, hc * 4 + b, qt * P:(qt + 1) * P]
                if b % 2 == 0:
                    nc.scalar.copy(dst, pst[:])
                else:
                    nc.vector.tensor_copy(dst, pst[:])

        prevq = None
        for hc in range(HC):
            wqc = wstream.tile([P, DT, 512], bf16, name="wchunk")
            nc.sync.dma_start(
                wqc[:],
                wq_d[:, hc * 512:(hc + 1) * 512].rearrange("(t p) o -> p t o", p=P))
            for qt in range(NQT):
                ps = pp.tile([P, 512], f32, name="pj")
                for dt in range(DT):
                    nc.tensor.matmul(ps[:], xq_sb[:, dt, qt * P:(qt + 1) * P],
                                     wqc[:, dt, :], start=(dt == 0),
                                     stop=(dt == DT - 1))
                if prevq is not None:
                    q_evict(*prevq)
                prevq = (ps, hc, qt)
        q_evict(*prevq)

        # ---- attention per tile t = heads (t, 16+t) ----
        rs_all = persist.tile([P, NQ], f32, name="rs_all")
        rs_rcp = persist.tile([P, NQ], bf16, name="rs_rcp")

        def norm_pass(trange, rows):
            with nc.allow_low_precision(reason="softmax denominator in bf16"):
                nc.vector.reciprocal(rs_rcp[rows], rs_all[rows])
            for t2 in trange:
                for par in range(2):
                    h_lo = par * 64
                    r = (t2 // 4) * 32 + (t2 % 4) * 2 + par
                    rcp0 = work.tile([1, NQ], bf16, name="rcp0")
                    nc.sync.dma_start(rcp0[:], rs_rcp[r:r + 1, :])
                    for (qc, qcw) in qchunks:
                        psBt = pp.tile([P, 512], f32, name="pj")[0:64, :qcw]
                        nc.tensor.matmul(psBt, ones64[:], rcp0[:, qc:qc + qcw],
                                         start=True, stop=True)
                        sl = attnT[h_lo:h_lo + 64, t2, qc:qc + qcw]
                        nc.vector.tensor_mul(sl, sl, psBt)

        for t in range(HQT):
            groups = (t // 4, 4 + t // 4)
            for (qc, qcw) in qchunks:
                live = [kt for kt in range(NKT)
                        if kt * P <= qc + qcw - 1 + offs_max]
                psO = [pv.tile([P, 512], f32, name="pvo")[:, :qcw]
                       for _ in range(2)]
                pms = {}

                def qk_exp_mask(kt, par):
                    lo = max(0, kt * P - qc - offs_max)
                    g = groups[par]
                    psS = psc.tile([P, 512], f32, name="psS")[:, :qcw]
                    nc.tensor.matmul(
                        psS[:, lo:], KtRz[:, g, kt * P:(kt + 1) * P],
                        Qt[:, t, qc + lo:qc + qcw], start=True, stop=True)
                    if causal:
                        pm = pmpool.tile([P, 512], bf16, name="pm")[:, :qcw]
                        nc.scalar.activation(pm[:, lo:], psS[:, lo:], EXPF,
                                             bias=0.0, scale=0.125)
                        d0 = kt * P - qc
                        dlo, dhi = max(lo, d0), min(qcw, d0 + P)
                        if dlo < dhi:
                            nc.vector.tensor_mul(
                                pm[:, dlo:dhi], pm[:, dlo:dhi],
                                mask_sb[:, kt, qc + dlo:qc + dhi])
                    else:
                        pexp = pmpool.tile([P, 512], bf16, name="pexp")[:, :qcw]
                        nc.scalar.activation(pexp[:, lo:], psS[:, lo:], EXPF,
                                             bias=0.0, scale=0.125)
                        pm = pmpool.tile([P, 512], bf16, name="pm")[:, :qcw]
                        nc.vector.tensor_mul(pm[:, lo:], pexp[:, lo:],
                                             mask_sb[:, kt, qc + lo:qc + qcw])
                    return pm, lo

                def pv_mm(idx):
                    kt = live[idx]
                    for par in range(2):
                        pm, lo = pms[(idx, par)]
                        nc.tensor.matmul(
                            psO[par][:, lo:], Vaug[:, kt, groups[par], :],
                            pm[:, lo:], start=(idx == 0),
                            stop=(idx == len(live) - 1), skip_group_check=True)

                for idx, kt in enumerate(live):
                    for par in range(2):
                        pms[(idx, par)] = qk_exp_mask(kt, par)
                    if idx > 0:
                        pv_mm(idx - 1)
                        del pms[(idx - 1, 0)], pms[(idx - 1, 1)]
                pv_mm(len(live) - 1)

                for par in range(2):
                    h_lo = par * 64
                    dst = attnT[h_lo:h_lo + 64, t, qc:qc + qcw]
                    if par == 0:
                        nc.scalar.copy(dst, psO[par][0:64, :])
                    else:
                        nc.vector.tensor_copy(dst, psO[par][0:64, :])
                    rsum0 = work.tile([1, 512], f32, name="rsum0")[:, :qcw]
                    nc.vector.tensor_copy(rsum0, psO[par][64:65, :])
                    r = (t // 4) * 32 + (t % 4) * 2 + par
                    nc.sync.dma_start(rs_all[r:r + 1, qc:qc + qcw], rsum0)
            if t % 4 == 3:
                qi = t // 4
                norm_pass(range(qi * 4, qi * 4 + 4), slice(qi * 32, qi * 32 + 8))

        # ---- output projection (wo rows head-permuted) ----
        for dc in range(4):
            woc = wstream.tile([P, DT, 512], bf16, name="wchunk")
            nc.sync.dma_start(
                woc[:],
                wo_d[:, dc * 512:(dc + 1) * 512].rearrange("(t p) o -> p t o", p=P))
            for qt in range(NQT):
                ps = pp.tile([P, 512], f32, name="pj")
                for j in range(HQT):
                    nc.tensor.matmul(ps[:], attnT[:, j, qt * P:(qt + 1) * P],
                                     woc[:, j, :], start=(j == 0),
                                     stop=(j == HQT - 1))
                osb = work.tile([P, 512], f32, name="osb")
                nc.vector.tensor_copy(osb[:], ps[:])
                nc.sync.dma_start(
                    out_d[qt * P:(qt + 1) * P, dc * 512:(dc + 1) * 512], osb[:])

    nc.finalize()
    _BUILD_CACHE[key] = nc
    return nc


# ---------------------------------------------------------------------------
# entry point
# ---------------------------------------------------------------------------

def kernel(x, freqs_cis, sequence_id, wq, wk, wv, wo):
    x = np.asarray(x, dtype=np.float32)
    freqs_cis = np.asarray(freqs_cis, dtype=np.float32)
    sequence_id = np.asarray(sequence_id)

    jobs = _plan_jobs(sequence_id)
    NQ = _round_up(max(max(j[2] for j in jobs), 1), P)
    NK = _round_up(max(max(j[1] + j[2] - j[3] for j in jobs), 1), P)
    offs_max = max(j[1] - j[3] for j in jobs)

    def single_doc(j):
        b, qs, ql, ks = j
        if ql == 0:
            return True
        seg = np.asarray(sequence_id[b])[ks:qs + ql]
        return bool((seg == seg[0]).all())

    causal = offs_max == 0 and all(single_doc(j) for j in jobs)

    cos_tab = freqs_cis[:, :, 0].astype(np.float32)
    sin_tab = freqs_cis[:, :, 1].astype(np.float32)
    wq_t = _permute_wq(np.ascontiguousarray(np.asarray(wq, np.float32).T)).astype(bf)
    wk_t = np.ascontiguousarray(np.asarray(wk, np.float32).T).astype(bf)
    wv_t = np.ascontiguousarray(np.asarray(wv, np.float32).T).astype(bf)
    wo_t = _permute_wo(np.ascontiguousarray(np.asarray(wo, np.float32).T)).astype(bf)
    id16 = np.eye(P, dtype=bf)

    in_maps = []
    for job in jobs:
        p = _core_inputs(job, NQ, NK, x, sequence_id, cos_tab, sin_tab)
        p.update({"wq_t": wq_t, "wk_t": wk_t, "wv_t": wv_t, "wo_t": wo_t,
                  "ident": id16})
        in_maps.append(p)

    nc = _build(NQ, NK, offs_max, causal)
    res = run_bass_kernel_spmd(nc, in_maps, core_ids=list(range(N_CORES)))

    full = np.zeros((BS, S, HQ), dtype=np.float32)
    for job, r in zip(jobs, res.results):
        b, qs, ql, ks = job
        if ql > 0:
            full[b, qs:qs + ql] = r["out"][:ql]
    return full
